# revision 14
# baseline (speedup 1.0000x reference)
"""3x3 zero-padded median filter (kornia MedianBlur semantics) on 8 trn2 cores.

Input  noised: (16, 3, 512, 512) f32, cover: same shape (pass-through).
Output (filtered, cover) — filtered is float32.

Sharding: pure data parallel over the 48 (B*C) images, 6 images per core.
Host packs each core's 6 images into one zero-separated stack I[3204, 514]
(one zero row between/around images gives the vertical zero padding; one
zero column each side gives the horizontal padding).  Partition p owns
R=25 consecutive output rows of the stack; the input window is 27 rows.

Algorithm (all DVE tensor_tensor min/max, fp16 2x mode, every operand
4-byte aligned):
  1. H-sort: per input row, sort each horizontal triple.  The three
     column taps come from three DMA loads of the same stack at column
     offsets 0/1/2 — no misaligned +1 reads, no shift copies.
       mH=min(t1,t2) MH=max(t1,t2); lo=min(t0,mH) hi=max(t0,MH)
       mid=max(min(t0,MH),mH)                       -> 6 ops/px
  2. V-merge: median9 = med3(max3(lo), med3(mid), min3(hi)) over the
     three vertical neighbors.  Vertical pairs live at row strides, so
     the half-rate shared-pair trick is alignment-free: pairs at odd
     slots s, even output r uses pair(r+1), odd r uses pair(r).
       pairs 4 ops at half rate + maxlo/minhi 1+1 + medmid 2 + final
       med3 4                                       -> 10 ops/px

Internal dtype float16 (exact median of fp16-rounded inputs; output
error ~= fp16 rounding, rel err ~2e-4).
"""

import numpy as np

import bass_rust
import concourse.bacc as bacc
import concourse.mybir as mybir
from concourse.tile import TileContext
from concourse.bass_utils import run_bass_kernel_spmd

B, CH, H, W = 16, 3, 512, 512
N_CORES = 8
IMGS = (B * CH) // N_CORES        # 6 images per core
SEP = H + 1                        # 513: image rows + 1 zero separator row
R = 25                             # output rows per partition (128*25 = 3200)
WP = W + 2                         # 514: padded input row width
WO = 512                           # output row width
IN_ROWS = 3204                     # 25*127 + 27 = 3202, zero padded
OUT_ROWS = 128 * R                 # 3200
INW = 27                           # input rows resident per partition

LOAD_CHUNKS = [(0, 2), (2, 5), (7, 5), (12, 5), (17, 5), (22, 5)]   # 27 rows
MERGE_CHUNKS = [(0, 12), (12, 13)]                                  # b even, 25 rows

MN = mybir.AluOpType.min
MX = mybir.AluOpType.max

NP_DT = np.float16

_CACHE = {}


def _view(tile, r0, n, width, col0=0, rowstride=WO):
    """AP over `n` rows (stride `rowstride`) of `tile`, cols [col0, col0+width)."""
    ap = tile[:, r0 * rowstride + col0: r0 * rowstride + col0 + width].copy()
    ap.ap = bass_rust.VecI64Pair([list(ap.ap[0]), [rowstride, n], [1, width]])
    return ap


def _build():
    if "nc" in _CACHE:
        return _CACHE["nc"]
    dt = mybir.dt.float16
    nc = bacc.Bacc(enable_partition_id=False)
    xin = nc.dram_tensor("xin", [IN_ROWS, WP], dt, kind="ExternalInput")
    yout = nc.dram_tensor("yout", [OUT_ROWS, WO], dt, kind="ExternalOutput")

    with TileContext(nc) as tc:
        with tc.tile_pool(name="p", bufs=1) as pool, \
             tc.tile_pool(name="ti", bufs=2) as tip, \
             tc.tile_pool(name="io", bufs=1) as iop:
            # full-height sorted-column planes
            L = pool.tile([128, INW * WO], dt, tag="L")
            Hh = pool.tile([128, INW * WO], dt, tag="H")
            M = pool.tile([128, INW * WO], dt, tag="M")

            MHs = pool.tile([128, 5 * WO], dt, tag="MHs")

            def load(a, n):
                """three column taps of stack rows [a, a+n) -> tin0/1/2.
                Issue on three different engine queues so the descriptor
                generation doesn't serialize on the sync engine."""
                tins = []
                for k, eng in ((0, nc.sync), (1, nc.scalar), (2, nc.gpsimd)):
                    t = tip.tile([128, 5 * WO], dt, tag=f"tin{k}")
                    ap = xin[0:1, 0:1].copy()
                    ap.ap = bass_rust.VecI64Pair([[R * WP, 128], [WP, n], [1, WO]])
                    ap.offset = a * WP + k
                    eng.dma_start(_view(t, 0, n, WO), ap)
                    tins.append(t)
                return tins

            def hsort(tins, a, n):
                t0, t1, t2 = (_view(t, 0, n, WO) for t in tins)
                Mv = _view(MHs, 0, n, WO)
                nc.vector.tensor_tensor(Mv, t1, t2, MX)
                m = t1                      # min pair in-place over tin1
                nc.vector.tensor_tensor(m, t1, t2, MN)
                nc.vector.tensor_tensor(_view(L, a, n, WO), t0, m, MN)
                nc.vector.tensor_tensor(_view(Hh, a, n, WO), t0, Mv, MX)
                te = _view(M, a, n, WO)
                nc.vector.tensor_tensor(te, t0, Mv, MN)
                nc.vector.tensor_tensor(te, te, m, MX)

            def merge(b, C):
                ne = (C + 1) // 2          # even outputs  r = b, b+2, ..
                no = C // 2                # odd outputs   r = b+1, b+3, ..
                npr = ne                   # pair slots s = b+1, b+3, ..
                # pairs tile also hosts t1 (rows 0..C) after pairs die
                prs = pool.tile([128, 4 * 7 * WO], dt, tag="prs")
                PL = lambda j0=0, n=npr: _view(prs, 0 * 7 + j0, n, WO)
                PH = lambda j0=0, n=npr: _view(prs, 1 * 7 + j0, n, WO)
                PN = lambda j0=0, n=npr: _view(prs, 2 * 7 + j0, n, WO)
                PX = lambda j0=0, n=npr: _view(prs, 3 * 7 + j0, n, WO)

                def odd(t, base, cnt):     # rows base, base+2, .. of plane t
                    return _view(t, 0, cnt, WO, base * WO, 2 * WO)

                nc.vector.tensor_tensor(PL(), odd(L, b + 1, npr), odd(L, b + 2, npr), MX)
                nc.vector.tensor_tensor(PH(), odd(Hh, b + 1, npr), odd(Hh, b + 2, npr), MN)
                nc.vector.tensor_tensor(PN(), odd(M, b + 1, npr), odd(M, b + 2, npr), MN)
                nc.vector.tensor_tensor(PX(), odd(M, b + 1, npr), odd(M, b + 2, npr), MX)

                ml = pool.tile([128, 13 * WO], dt, tag="ml")
                mh = pool.tile([128, 13 * WO], dt, tag="mh")
                md = pool.tile([128, 13 * WO], dt, tag="md")
                tS = pool.tile([128, 7 * WO], dt, tag="tS")
                out = iop.tile([128, 13 * WO], dt, tag="out")

                # even outputs: single = row r, pair slot s = r+1 (index j)
                nc.vector.tensor_tensor(odd(ml, 0, ne), odd(L, b, ne), PL(0, ne), MX)
                nc.vector.tensor_tensor(odd(mh, 0, ne), odd(Hh, b, ne), PH(0, ne), MN)
                nc.vector.tensor_tensor(_view(tS, 0, ne, WO), odd(M, b, ne), PX(0, ne), MN)
                nc.vector.tensor_tensor(odd(md, 0, ne), _view(tS, 0, ne, WO), PN(0, ne), MX)
                if no:
                    # odd outputs: single = row r+2, pair slot s = r (index j)
                    nc.vector.tensor_tensor(odd(ml, 1, no), PL(0, no), odd(L, b + 3, no), MX)
                    nc.vector.tensor_tensor(odd(mh, 1, no), PH(0, no), odd(Hh, b + 3, no), MN)
                    nc.vector.tensor_tensor(_view(tS, 0, no, WO), odd(M, b + 3, no), PX(0, no), MN)
                    nc.vector.tensor_tensor(odd(md, 1, no), _view(tS, 0, no, WO), PN(0, no), MX)

                # final med3(ml, md, mh); t1 reuses the pairs tile rows
                mlv = _view(ml, 0, C, WO)
                mdv = _view(md, 0, C, WO)
                mhv = _view(mh, 0, C, WO)
                t1 = _view(prs, 0, C, WO)
                nc.vector.tensor_tensor(t1, mlv, mdv, MN)
                nc.vector.tensor_tensor(mlv, mlv, mdv, MX)
                nc.vector.tensor_tensor(mlv, mlv, mhv, MN)
                nc.vector.tensor_tensor(_view(out, 0, C, WO), t1, mlv, MX)

                dst = yout[0:1, 0:1].copy()
                dst.ap = bass_rust.VecI64Pair([[R * WO, 128], [WO, C], [1, WO]])
                dst.offset = b * WO
                nc.sync.dma_start(dst, _view(out, 0, C, WO))

            # software pipeline: keep loads one chunk ahead of the sorts,
            # merge0 (rows 0..13) after sorts 0-3, merge1 (12..26) at the end
            tins = {}
            tins[0] = load(*LOAD_CHUNKS[0])
            tins[1] = load(*LOAD_CHUNKS[1])
            hsort(tins.pop(0), *LOAD_CHUNKS[0])
            tins[2] = load(*LOAD_CHUNKS[2])
            hsort(tins.pop(1), *LOAD_CHUNKS[1])
            tins[3] = load(*LOAD_CHUNKS[3])
            hsort(tins.pop(2), *LOAD_CHUNKS[2])
            tins[4] = load(*LOAD_CHUNKS[4])
            hsort(tins.pop(3), *LOAD_CHUNKS[3])
            tins[5] = load(*LOAD_CHUNKS[5])
            merge(*MERGE_CHUNKS[0])
            hsort(tins.pop(4), *LOAD_CHUNKS[4])
            hsort(tins.pop(5), *LOAD_CHUNKS[5])
            merge(*MERGE_CHUNKS[1])

    nc.compile()
    _CACHE["nc"] = nc
    return nc


def _pack(core_imgs):
    """core_imgs: (IMGS, H, W) -> I[IN_ROWS, WP] in the device dtype."""
    I = np.zeros((IN_ROWS, WP), NP_DT)
    for i in range(IMGS):
        r0 = 1 + i * SEP
        I[r0: r0 + H, 1: 1 + W] = core_imgs[i].astype(NP_DT)
    return I


def _in_maps(noised):
    imgs = np.asarray(noised, dtype=np.float32).reshape(B * CH, H, W)
    return [{"xin": _pack(imgs[c * IMGS:(c + 1) * IMGS])} for c in range(N_CORES)]


def kernel(noised, cover):
    cover = np.asarray(cover)
    nc = _build()
    in_maps = _in_maps(noised)
    res = run_bass_kernel_spmd(nc, in_maps, core_ids=list(range(N_CORES)))
    out = np.empty((B * CH, H, W), np.float32)
    for c in range(N_CORES):
        Y = res.results[c]["yout"]
        for i in range(IMGS):
            out[c * IMGS + i] = Y[i * SEP: i * SEP + H, :].astype(np.float32)
    filtered = out.reshape(B, CH, H, W)
    return filtered, cover


# revision 17
# speedup vs baseline: 1.0033x; 1.0033x over previous
"""3x3 zero-padded median filter (kornia MedianBlur semantics) on 8 trn2 cores.

Input  noised: (16, 3, 512, 512) f32, cover: same shape (pass-through).
Output (filtered, cover) — filtered is float32.

Sharding: pure data parallel over the 48 (B*C) images, 6 images per core.
Host packs each core's 6 images into one zero-separated stack I[3204, 514]
(one zero row between/around images gives the vertical zero padding; one
zero column each side gives the horizontal padding).  Partition p owns
R=25 consecutive output rows of the stack; the input window is 27 rows.

Algorithm (all DVE tensor_tensor min/max, fp16 2x mode, every operand
4-byte aligned):
  1. H-sort: per input row, sort each horizontal triple.  The three
     column taps come from three DMA loads of the same stack at column
     offsets 0/1/2 — no misaligned +1 reads, no shift copies.
       mH=min(t1,t2) MH=max(t1,t2); lo=min(t0,mH) hi=max(t0,MH)
       mid=max(min(t0,MH),mH)                       -> 6 ops/px
  2. V-merge: median9 = med3(max3(lo), med3(mid), min3(hi)) over the
     three vertical neighbors.  Vertical pairs live at row strides, so
     the half-rate shared-pair trick is alignment-free: pairs at odd
     slots s, even output r uses pair(r+1), odd r uses pair(r).
       pairs 4 ops at half rate + maxlo/minhi 1+1 + medmid 2 + final
       med3 4                                       -> 10 ops/px

Internal dtype float16 (exact median of fp16-rounded inputs; output
error ~= fp16 rounding, rel err ~2e-4).
"""

import numpy as np

import bass_rust
import concourse.bacc as bacc
import concourse.mybir as mybir
from concourse.tile import TileContext
from concourse.bass_utils import run_bass_kernel_spmd

B, CH, H, W = 16, 3, 512, 512
N_CORES = 8
IMGS = (B * CH) // N_CORES        # 6 images per core
SEP = H + 1                        # 513: image rows + 1 zero separator row
R = 25                             # output rows per partition (128*25 = 3200)
WP = W + 2                         # 514: padded input row width
WO = 512                           # output row width
IN_ROWS = 3204                     # 25*127 + 27 = 3202, zero padded
OUT_ROWS = 128 * R                 # 3200
INW = 27                           # input rows resident per partition

LOAD_CHUNKS = [(0, 3), (3, 6), (9, 6), (15, 6), (21, 6)]   # 27 rows
MERGE_CHUNKS = [(0, 12), (12, 13)]                         # b even, 25 rows

MN = mybir.AluOpType.min
MX = mybir.AluOpType.max

NP_DT = np.float16

_CACHE = {}


def _view(tile, r0, n, width, col0=0, rowstride=WO):
    """AP over `n` rows (stride `rowstride`) of `tile`, cols [col0, col0+width)."""
    ap = tile[:, r0 * rowstride + col0: r0 * rowstride + col0 + width].copy()
    ap.ap = bass_rust.VecI64Pair([list(ap.ap[0]), [rowstride, n], [1, width]])
    return ap


def _view4(tile, off, pstride, rowstride, n, width):
    """4D AP: two planes (stride `pstride`) x n rows x width, from elem `off`."""
    ap = tile[:, off: off + width].copy()
    ap.ap = bass_rust.VecI64Pair(
        [list(ap.ap[0]), [pstride, 2], [rowstride, n], [1, width]])
    return ap


def _build():
    if "nc" in _CACHE:
        return _CACHE["nc"]
    dt = mybir.dt.float16
    nc = bacc.Bacc(enable_partition_id=False)
    xin = nc.dram_tensor("xin", [IN_ROWS, WP], dt, kind="ExternalInput")
    yout = nc.dram_tensor("yout", [OUT_ROWS, WO], dt, kind="ExternalOutput")

    PP = INW * WO                  # plane stride inside the lmh super-tile

    with TileContext(nc) as tc:
        with tc.tile_pool(name="p", bufs=1) as pool, \
             tc.tile_pool(name="ti", bufs=2) as tip:
            # sorted-column planes [L, M, H] in one super-tile so pair ops can
            # span two planes with a 4D access pattern (one instruction each)
            lmh = pool.tile([128, 3 * PP], dt, tag="lmh")
            MHs = pool.tile([128, 6 * WO], dt, tag="MHs")

            def plane(p, base, cnt, stride=WO):   # rows base.. of plane p
                return _view(lmh, 0, cnt, WO, (p * INW + base) * WO, stride)

            def load(a, n):
                """three column taps of stack rows [a, a+n) -> tin0/1/2.
                Issue on three different engine queues so the descriptor
                generation doesn't serialize on the sync engine."""
                tins = []
                for k, eng in ((0, nc.sync), (1, nc.scalar), (2, nc.gpsimd)):
                    t = tip.tile([128, 6 * WO], dt, tag=f"tin{k}")
                    ap = xin[0:1, 0:1].copy()
                    ap.ap = bass_rust.VecI64Pair([[R * WP, 128], [WP, n], [1, WO]])
                    ap.offset = a * WP + k
                    eng.dma_start(_view(t, 0, n, WO), ap)
                    tins.append(t)
                return tins

            def hsort(tins, a, n):
                t0, t1, t2 = (_view(t, 0, n, WO) for t in tins)
                Mv = _view(MHs, 0, n, WO)
                nc.vector.tensor_tensor(Mv, t1, t2, MX)
                m = t1                      # min pair in-place over tin1
                nc.vector.tensor_tensor(m, t1, t2, MN)
                nc.vector.tensor_tensor(plane(0, a, n), t0, m, MN)
                nc.vector.tensor_tensor(plane(2, a, n), t0, Mv, MX)
                te = plane(1, a, n)
                nc.vector.tensor_tensor(te, t0, Mv, MN)
                nc.vector.tensor_tensor(te, te, m, MX)

            def merge(b, C, split_tail=False):
                ne = (C + 1) // 2          # even outputs  r = b, b+2, ..
                no = C // 2                # odd outputs   r = b+1, b+3, ..
                npr = ne                   # pair slots s = b+1, b+3, ..
                # pair regions [PL, PX, PN, PH]; t1 reuses rows 0..C later
                prs = pool.tile([128, 4 * 7 * WO], dt, tag="prs")
                PL = lambda j0, n: _view(prs, 0 * 7 + j0, n, WO)
                PX = lambda j0, n: _view(prs, 1 * 7 + j0, n, WO)
                PN = lambda j0, n: _view(prs, 2 * 7 + j0, n, WO)
                PH = lambda j0, n: _view(prs, 3 * 7 + j0, n, WO)

                # fused pairs: {PL,PX} = max over planes {L,M}; {PN,PH} = min
                # over planes {M,H} at odd slots s, s+1
                nc.vector.tensor_tensor(
                    _view4(prs, 0, 7 * WO, WO, npr, WO),
                    _view4(lmh, (b + 1) * WO, PP, 2 * WO, npr, WO),
                    _view4(lmh, (b + 2) * WO, PP, 2 * WO, npr, WO), MX)
                nc.vector.tensor_tensor(
                    _view4(prs, 2 * 7 * WO, 7 * WO, WO, npr, WO),
                    _view4(lmh, PP + (b + 1) * WO, PP, 2 * WO, npr, WO),
                    _view4(lmh, PP + (b + 2) * WO, PP, 2 * WO, npr, WO), MN)

                ml = pool.tile([128, 13 * WO], dt, tag="ml")
                mh = pool.tile([128, 13 * WO], dt, tag="mh")
                md = pool.tile([128, 13 * WO], dt, tag="md")
                tS = pool.tile([128, 7 * WO], dt, tag="tS")

                def odd(t, base, cnt):     # rows base, base+2, .. of tile t
                    return _view(t, 0, cnt, WO, base * WO, 2 * WO)

                # even outputs: single = row r, pair slot s = r+1 (index j)
                nc.vector.tensor_tensor(odd(ml, 0, ne), plane(0, b, ne, 2 * WO), PL(0, ne), MX)
                nc.vector.tensor_tensor(odd(mh, 0, ne), plane(2, b, ne, 2 * WO), PH(0, ne), MN)
                nc.vector.tensor_tensor(_view(tS, 0, ne, WO), plane(1, b, ne, 2 * WO), PX(0, ne), MN)
                nc.vector.tensor_tensor(odd(md, 0, ne), _view(tS, 0, ne, WO), PN(0, ne), MX)
                # odd outputs: single = row r+2, pair slot s = r (index j)
                nc.vector.tensor_tensor(odd(ml, 1, no), PL(0, no), plane(0, b + 3, no, 2 * WO), MX)
                nc.vector.tensor_tensor(odd(mh, 1, no), PH(0, no), plane(2, b + 3, no, 2 * WO), MN)
                nc.vector.tensor_tensor(_view(tS, 0, no, WO), plane(1, b + 3, no, 2 * WO), PX(0, no), MN)
                nc.vector.tensor_tensor(odd(md, 1, no), _view(tS, 0, no, WO), PN(0, no), MX)

                # final med3(ml, md, mh) -> md; t1 reuses the pairs tile rows.
                # split_tail: two halves so the first store overlaps the rest.
                pieces = [(0, C)] if not split_tail else [(0, C // 2), (C // 2, C - C // 2)]
                for (o, n) in pieces:
                    mlv = _view(ml, o, n, WO)
                    mdv = _view(md, o, n, WO)
                    mhv = _view(mh, o, n, WO)
                    t1 = _view(prs, o, n, WO)
                    nc.vector.tensor_tensor(t1, mlv, mdv, MN)
                    nc.vector.tensor_tensor(mlv, mlv, mdv, MX)
                    nc.vector.tensor_tensor(mlv, mlv, mhv, MN)
                    nc.vector.tensor_tensor(mdv, t1, mlv, MX)
                    dst = yout[0:1, 0:1].copy()
                    dst.ap = bass_rust.VecI64Pair([[R * WO, 128], [WO, n], [1, WO]])
                    dst.offset = (b + o) * WO
                    nc.sync.dma_start(dst, mdv)

            # software pipeline: keep loads one chunk ahead of the sorts,
            # merge0 (rows 0..13) after sorts 0-3, merge1 (12..26) at the end
            tins = {}
            tins[0] = load(*LOAD_CHUNKS[0])
            tins[1] = load(*LOAD_CHUNKS[1])
            hsort(tins.pop(0), *LOAD_CHUNKS[0])
            tins[2] = load(*LOAD_CHUNKS[2])
            hsort(tins.pop(1), *LOAD_CHUNKS[1])
            tins[3] = load(*LOAD_CHUNKS[3])
            hsort(tins.pop(2), *LOAD_CHUNKS[2])
            tins[4] = load(*LOAD_CHUNKS[4])
            hsort(tins.pop(3), *LOAD_CHUNKS[3])
            merge(*MERGE_CHUNKS[0])
            hsort(tins.pop(4), *LOAD_CHUNKS[4])
            merge(*MERGE_CHUNKS[1], split_tail=True)

    nc.compile()
    _CACHE["nc"] = nc
    return nc


def _pack(core_imgs):
    """core_imgs: (IMGS, H, W) -> I[IN_ROWS, WP] in the device dtype."""
    I = np.zeros((IN_ROWS, WP), NP_DT)
    for i in range(IMGS):
        r0 = 1 + i * SEP
        I[r0: r0 + H, 1: 1 + W] = core_imgs[i].astype(NP_DT)
    return I


def _in_maps(noised):
    imgs = np.asarray(noised, dtype=np.float32).reshape(B * CH, H, W)
    return [{"xin": _pack(imgs[c * IMGS:(c + 1) * IMGS])} for c in range(N_CORES)]


def kernel(noised, cover):
    cover = np.asarray(cover)
    nc = _build()
    in_maps = _in_maps(noised)
    res = run_bass_kernel_spmd(nc, in_maps, core_ids=list(range(N_CORES)))
    out = np.empty((B * CH, H, W), np.float32)
    for c in range(N_CORES):
        Y = res.results[c]["yout"]
        for i in range(IMGS):
            out[c * IMGS + i] = Y[i * SEP: i * SEP + H, :].astype(np.float32)
    filtered = out.reshape(B, CH, H, W)
    return filtered, cover


# revision 20
# speedup vs baseline: 1.0276x; 1.0242x over previous
"""3x3 zero-padded median filter (kornia MedianBlur semantics) on 8 trn2 cores.

Input  noised: (16, 3, 512, 512) f32, cover: same shape (pass-through).
Output (filtered, cover) — filtered is float32.

Sharding: pure data parallel over the 48 (B*C) images, 6 images per core.
Host packs each core's 6 images into one zero-separated stack of 3204 rows
(one zero row between/around images gives the vertical zero padding), with
columns DEINTERLEAVED into even/odd planes: row = [Pe (258 cols) | Po
(258 cols)] where Pe[u] = padded-row[2u], Po[u] = padded-row[2u+1].
Partition p owns R=25 consecutive output rows; input window is 27 rows.

All compute is DVE tensor_tensor min/max in fp16 2x mode, every operand
4-byte aligned.  The even/odd split makes the horizontal shared pair
(columns x+1, x+2, shared between triples x and x+1) an aligned operand:

  1. H-sort, 6 instructions per row chunk (5 ops/px):
       mHe=min(Po0,Pe1)  MHe=max(Po0,Pe1)       pair, shared by parities
       {Le,Lo}=min(({Pe0,Po1}), mHe)            fused across parity
       {He,Ho}=max(({Pe0,Po1}), MHe)
       {Me,Mo}=max(min({Pe0,Po1}, MHe), mHe)
  2. V-merge (10 ops/px): median9 = med3(max3(lo), med3(mid), min3(hi))
     over vertical neighbors; vertical pairs at odd row slots are computed
     once and shared by the two adjacent output rows (even output r uses
     pair(r+1), odd r uses pair(r)); pair instructions span 4 planes.

Output is stored column-deinterleaved ([even 256 | odd 256] per row); the
host re-interleaves.  Internal dtype float16 (exact median of fp16-rounded
inputs; output error ~= fp16 rounding, rel err ~2e-4).
"""

import numpy as np

import bass_rust
import concourse.bacc as bacc
import concourse.mybir as mybir
from concourse.tile import TileContext
from concourse.bass_utils import run_bass_kernel_spmd

B, CH, H, W = 16, 3, 512, 512
N_CORES = 8
IMGS = (B * CH) // N_CORES        # 6 images per core
SEP = H + 1                        # 513: image rows + 1 zero separator row
R = 25                             # output rows per partition (128*25 = 3200)
W2 = 256                           # half output width (one parity)
WE = 258                           # stored width of one input parity plane
WP = 2 * WE                        # 516: packed input row width
WO = 512                           # output row width
IN_ROWS = 3204                     # 25*127 + 27 = 3202, zero padded
OUT_ROWS = 128 * R                 # 3200
INW = 27                           # input rows resident per partition

LOAD_CHUNKS = [(0, 1), (1, 7), (8, 7), (15, 6), (21, 6)]   # 27 rows
MERGE_CHUNKS = [(0, 12), (12, 13)]                         # b even, 25 rows

MN = mybir.AluOpType.min
MX = mybir.AluOpType.max

NP_DT = np.float16

_CACHE = {}

PP = INW * W2                  # plane stride inside the lmh super-tile


def _view(tile, r0, n, width, col0=0, rowstride=W2):
    """AP over `n` rows (stride `rowstride`) of `tile`, cols [col0, col0+width)."""
    ap = tile[:, r0 * rowstride + col0: r0 * rowstride + col0 + width].copy()
    ap.ap = bass_rust.VecI64Pair([list(ap.ap[0]), [rowstride, n], [1, width]])
    return ap


def _viewp(tile, off, nplanes, pstride, n, rowstride, width=W2):
    """4D AP: nplanes planes x n rows x width elems, from elem offset `off`."""
    ap = tile[:, off: off + width].copy()
    ap.ap = bass_rust.VecI64Pair(
        [list(ap.ap[0]), [pstride, nplanes], [rowstride, n], [1, width]])
    return ap


def _build():
    if "nc" in _CACHE:
        return _CACHE["nc"]
    dt = mybir.dt.float16
    nc = bacc.Bacc(enable_partition_id=False)
    xin = nc.dram_tensor("xin", [IN_ROWS, WP], dt, kind="ExternalInput")
    yout = nc.dram_tensor("yout", [OUT_ROWS, WO], dt, kind="ExternalOutput")

    with TileContext(nc) as tc:
        with tc.tile_pool(name="p", bufs=1) as pool, \
             tc.tile_pool(name="ti", bufs=2) as tip:
            # sorted-column planes [Le, Lo, Me, Mo, He, Ho] in one super-tile
            # so pair/parity ops span planes with one 4D access pattern
            lmh = pool.tile([128, 6 * PP], dt, tag="lmh")
            mHe = pool.tile([128, 7 * W2], dt, tag="mHe")
            MHe = pool.tile([128, 7 * W2], dt, tag="MHe")

            def planes(p0, np_, base, cnt, stride=W2):
                return _viewp(lmh, (p0 * INW + base) * W2, np_, PP, cnt, stride)

            def load(a, n):
                """4 taps of stack rows [a, a+n): Tsingle = [Pe0; Po1],
                Tpair = [Po0; Pe1], each two planes of n x 256.  Issue on
                different queues so descriptor generation overlaps."""
                ts = tip.tile([128, 2 * 7 * W2], dt, tag="ts")
                tp = tip.tile([128, 2 * 7 * W2], dt, tag="tp")
                for (t, pl, off, eng) in (
                        (ts, 0, 0, nc.sync),         # Pe0 = Pe[u]
                        (ts, 1, WE + 1, nc.scalar),  # Po1 = Po[u+1]
                        (tp, 0, WE, nc.gpsimd),      # Po0 = Po[u]
                        (tp, 1, 1, nc.sync)):        # Pe1 = Pe[u+1]
                    ap = xin[0:1, 0:1].copy()
                    ap.ap = bass_rust.VecI64Pair([[R * WP, 128], [WP, n], [1, W2]])
                    ap.offset = a * WP + off
                    eng.dma_start(_view(t, 0, n, W2, pl * 7 * W2), ap)
                return ts, tp

            def rep2(t, n):            # one plane broadcast to both parities
                return _viewp(t, 0, 2, 0, n, W2)

            def hsort(tins, a, n):
                ts, tp = tins
                p0 = _view(tp, 0, n, W2)              # Po0
                p1 = _view(tp, 0, n, W2, 7 * W2)      # Pe1
                m = _view(mHe, 0, n, W2)
                Mv = _view(MHe, 0, n, W2)
                nc.vector.tensor_tensor(m, p0, p1, MN)
                nc.vector.tensor_tensor(Mv, p0, p1, MX)
                sg = _viewp(ts, 0, 2, 7 * W2, n, W2)  # {Pe0, Po1}
                nc.vector.tensor_tensor(planes(0, 2, a, n), sg, rep2(mHe, n), MN)
                nc.vector.tensor_tensor(planes(4, 2, a, n), sg, rep2(MHe, n), MX)
                te = planes(2, 2, a, n)
                nc.vector.tensor_tensor(te, sg, rep2(MHe, n), MN)
                nc.vector.tensor_tensor(te, te, rep2(mHe, n), MX)

            def merge(b, C, split_tail=False):
                ne = (C + 1) // 2          # even outputs  r = b, b+2, ..
                no = C // 2                # odd outputs   r = b+1, b+3, ..
                npr = ne                   # pair slots s = b+1, b+3, ..
                # pair regions [PLe,PLo,PXe,PXo,PNe,PNo,PHe,PHo] of 7 rows
                prs = pool.tile([128, 8 * 7 * W2], dt, tag="prs")

                def P(reg, j0, n):         # two-parity pair view at region
                    return _viewp(prs, reg * 7 * W2, 2, 7 * W2, n, W2, W2)

                nc.vector.tensor_tensor(
                    _viewp(prs, 0, 4, 7 * W2, npr, W2),
                    planes(0, 4, b + 1, npr, 2 * W2),
                    planes(0, 4, b + 2, npr, 2 * W2), MX)
                nc.vector.tensor_tensor(
                    _viewp(prs, 4 * 7 * W2, 4, 7 * W2, npr, W2),
                    planes(2, 4, b + 1, npr, 2 * W2),
                    planes(2, 4, b + 2, npr, 2 * W2), MN)

                ml = pool.tile([128, 2 * 13 * W2], dt, tag="ml")
                mh = pool.tile([128, 2 * 13 * W2], dt, tag="mh")
                md = pool.tile([128, 2 * 13 * W2], dt, tag="md")
                tS = pool.tile([128, 2 * 7 * W2], dt, tag="tS")

                def out2(t, base, cnt, nrows=13):   # both parities, rows base, base+2..
                    return _viewp(t, base * W2, 2, nrows * W2, cnt, 2 * W2)

                def tSv(n):                # compact two-parity scratch view
                    return _viewp(tS, 0, 2, 7 * W2, n, W2)

                # even outputs: single = row r, pair slot s = r+1 (index j)
                nc.vector.tensor_tensor(out2(ml, 0, ne), planes(0, 2, b, ne, 2 * W2), P(0, 0, ne), MX)
                nc.vector.tensor_tensor(out2(mh, 0, ne), planes(4, 2, b, ne, 2 * W2), P(6, 0, ne), MN)
                nc.vector.tensor_tensor(tSv(ne), planes(2, 2, b, ne, 2 * W2), P(2, 0, ne), MN)
                nc.vector.tensor_tensor(out2(md, 0, ne), tSv(ne), P(4, 0, ne), MX)
                # odd outputs: single = row r+2, pair slot s = r (index j)
                nc.vector.tensor_tensor(out2(ml, 1, no), P(0, 0, no), planes(0, 2, b + 3, no, 2 * W2), MX)
                nc.vector.tensor_tensor(out2(mh, 1, no), P(6, 0, no), planes(4, 2, b + 3, no, 2 * W2), MN)
                nc.vector.tensor_tensor(tSv(no), planes(2, 2, b + 3, no, 2 * W2), P(2, 0, no), MN)
                nc.vector.tensor_tensor(out2(md, 1, no), tSv(no), P(4, 0, no), MX)

                # final med3(ml, md, mh) -> md; t1 reuses the pairs tile.
                # split_tail: two pieces so the first store overlaps the rest.
                pieces = [(0, C)] if not split_tail else [(0, C // 2), (C // 2, C - C // 2)]
                for (o, n) in pieces:
                    mlv = _viewp(ml, o * W2, 2, 13 * W2, n, W2)
                    mdv = _viewp(md, o * W2, 2, 13 * W2, n, W2)
                    mhv = _viewp(mh, o * W2, 2, 13 * W2, n, W2)
                    t1 = _viewp(prs, o * W2, 2, 13 * W2, n, W2)
                    nc.vector.tensor_tensor(t1, mlv, mdv, MN)
                    nc.vector.tensor_tensor(mlv, mlv, mdv, MX)
                    nc.vector.tensor_tensor(mlv, mlv, mhv, MN)
                    nc.vector.tensor_tensor(mdv, t1, mlv, MX)
                    for par, eng in ((0, nc.sync), (1, nc.scalar)):
                        dst = yout[0:1, 0:1].copy()
                        dst.ap = bass_rust.VecI64Pair(
                            [[R * WO, 128], [WO, n], [1, W2]])
                        dst.offset = (b + o) * WO + par * W2
                        eng.dma_start(dst, _view(md, o, n, W2, par * 13 * W2, W2))

            # software pipeline: loads one chunk ahead of the sorts,
            # merge0 (rows 0..13) after sorts 0-3, merge1 (12..26) last
            tins = {}
            tins[0] = load(*LOAD_CHUNKS[0])
            tins[1] = load(*LOAD_CHUNKS[1])
            hsort(tins.pop(0), *LOAD_CHUNKS[0])
            tins[2] = load(*LOAD_CHUNKS[2])
            hsort(tins.pop(1), *LOAD_CHUNKS[1])
            tins[3] = load(*LOAD_CHUNKS[3])
            hsort(tins.pop(2), *LOAD_CHUNKS[2])
            tins[4] = load(*LOAD_CHUNKS[4])
            hsort(tins.pop(3), *LOAD_CHUNKS[3])
            merge(*MERGE_CHUNKS[0])
            hsort(tins.pop(4), *LOAD_CHUNKS[4])
            merge(*MERGE_CHUNKS[1], split_tail=True)

    nc.compile()
    _CACHE["nc"] = nc
    return nc


def _pack(core_imgs):
    """core_imgs: (IMGS, H, W) -> I[IN_ROWS, WP] column-deinterleaved."""
    P = np.zeros((IN_ROWS, W + 2), NP_DT)
    for i in range(IMGS):
        r0 = 1 + i * SEP
        P[r0: r0 + H, 1: 1 + W] = core_imgs[i].astype(NP_DT)
    I = np.zeros((IN_ROWS, WP), NP_DT)
    I[:, 0:257] = P[:, 0::2]           # Pe: 257 of 258 cols used
    I[:, WE:WE + 257] = P[:, 1::2]     # Po
    return I


def _in_maps(noised):
    imgs = np.asarray(noised, dtype=np.float32).reshape(B * CH, H, W)
    return [{"xin": _pack(imgs[c * IMGS:(c + 1) * IMGS])} for c in range(N_CORES)]


def kernel(noised, cover):
    cover = np.asarray(cover)
    nc = _build()
    in_maps = _in_maps(noised)
    res = run_bass_kernel_spmd(nc, in_maps, core_ids=list(range(N_CORES)))
    out = np.empty((B * CH, H, W), np.float32)
    for c in range(N_CORES):
        Y = res.results[c]["yout"]
        for i in range(IMGS):
            Yi = Y[i * SEP: i * SEP + H]
            out[c * IMGS + i, :, 0::2] = Yi[:, :W2].astype(np.float32)
            out[c * IMGS + i, :, 1::2] = Yi[:, W2:].astype(np.float32)
    filtered = out.reshape(B, CH, H, W)
    return filtered, cover


# revision 25
# speedup vs baseline: 1.0370x; 1.0092x over previous
"""3x3 zero-padded median filter (kornia MedianBlur semantics) on 8 trn2 cores.

Input  noised: (16, 3, 512, 512) f32, cover: same shape (pass-through).
Output (filtered, cover) — filtered is float32.

Sharding: pure data parallel over the 48 (B*C) images, 6 images per core.
Host packs each core's 6 images into one zero-separated stack of 3204 rows
(one zero row between/around images gives the vertical zero padding), with
columns DEINTERLEAVED into even/odd planes: row = [Pe (258 cols) | Po
(258 cols)] where Pe[u] = padded-row[2u], Po[u] = padded-row[2u+1].
Partition p owns R=25 consecutive output rows; input window is 27 rows.

All compute is DVE tensor_tensor min/max in fp16 2x mode, every operand
4-byte aligned.  The even/odd split makes the horizontal shared pair
(columns x+1, x+2, shared between triples x and x+1) an aligned operand:

  1. H-sort, 6 instructions per row chunk (5 ops/px):
       mHe=min(Po0,Pe1)  MHe=max(Po0,Pe1)       pair, shared by parities
       {Le,Lo}=min(({Pe0,Po1}), mHe)            fused across parity
       {He,Ho}=max(({Pe0,Po1}), MHe)
       {Me,Mo}=max(min({Pe0,Po1}, MHe), mHe)
  2. V-merge (10 ops/px): median9 = med3(max3(lo), med3(mid), min3(hi))
     over vertical neighbors; vertical pairs at odd row slots are computed
     once and shared by the two adjacent output rows (even output r uses
     pair(r+1), odd r uses pair(r)); pair instructions span 4 planes.

Output is stored column-deinterleaved ([even 256 | odd 256] per row); the
host re-interleaves.  Internal dtype float16 (exact median of fp16-rounded
inputs; output error ~= fp16 rounding, rel err ~2e-4).
"""

import numpy as np

import bass_rust
import concourse.bacc as bacc
import concourse.mybir as mybir
from concourse.tile import TileContext
from concourse.bass_utils import run_bass_kernel_spmd

B, CH, H, W = 16, 3, 512, 512
N_CORES = 8
IMGS = (B * CH) // N_CORES        # 6 images per core
SEP = H + 1                        # 513: image rows + 1 zero separator row
R = 25                             # output rows per partition (128*25 = 3200)
W2 = 256                           # half output width (one parity)
WE = 258                           # stored width of one input parity plane
WP = 2 * WE                        # 516: packed input row width
WO = 512                           # output row width
IN_ROWS = 3204                     # 25*127 + 27 = 3202, zero padded
OUT_ROWS = 128 * R                 # 3200
INW = 27                           # input rows resident per partition

LOAD_CHUNKS = [(0, 6), (6, 7), (13, 7), (20, 7)]           # 27 rows
MERGE_CHUNKS = [(0, 12), (12, 13)]                         # b even, 25 rows
TAPE = INW * W2                                            # one tap-plane block

MN = mybir.AluOpType.min
MX = mybir.AluOpType.max

NP_DT = np.float16

_CACHE = {}

PP = INW * W2                  # plane stride inside the lmh super-tile


def _view(tile, r0, n, width, col0=0, rowstride=W2):
    """AP over `n` rows (stride `rowstride`) of `tile`, cols [col0, col0+width)."""
    ap = tile[:, r0 * rowstride + col0: r0 * rowstride + col0 + width].copy()
    ap.ap = bass_rust.VecI64Pair([list(ap.ap[0]), [rowstride, n], [1, width]])
    return ap


def _viewp(tile, off, nplanes, pstride, n, rowstride, width=W2):
    """4D AP: nplanes planes x n rows x width elems, from elem offset `off`."""
    ap = tile[:, off: off + width].copy()
    ap.ap = bass_rust.VecI64Pair(
        [list(ap.ap[0]), [pstride, nplanes], [rowstride, n], [1, width]])
    return ap


def _build():
    if "nc" in _CACHE:
        return _CACHE["nc"]
    dt = mybir.dt.float16
    nc = bacc.Bacc(enable_partition_id=False)
    # xin row (tap*128 + p) holds tap-plane [27 x 256] of partition p,
    # contiguous, so every load is a single linear burst per partition
    xin = nc.dram_tensor("xin", [4 * 128, TAPE], dt, kind="ExternalInput")
    yout = nc.dram_tensor("yout", [OUT_ROWS, WO], dt, kind="ExternalOutput")

    with TileContext(nc) as tc:
        with tc.tile_pool(name="p", bufs=1) as pool, \
             tc.tile_pool(name="ti", bufs=2) as tip:
            # sorted-column planes [Le, Lo, Me, Mo, He, Ho] in one super-tile
            # so pair/parity ops span planes with one 4D access pattern
            lmh = pool.tile([128, 6 * PP], dt, tag="lmh")
            mHe = pool.tile([128, 7 * W2], dt, tag="mHe")
            MHe = pool.tile([128, 7 * W2], dt, tag="MHe")

            def planes(p0, np_, base, cnt, stride=W2):
                return _viewp(lmh, (p0 * INW + base) * W2, np_, PP, cnt, stride)

            def load(a, n):
                """4 taps of stack rows [a, a+n): Tsingle = [Pe0; Po1],
                Tpair = [Po0; Pe1], each two planes of n x 256.  Issue on
                different queues so descriptor generation overlaps."""
                ts = tip.tile([128, 2 * 7 * W2], dt, tag="ts")
                tp = tip.tile([128, 2 * 7 * W2], dt, tag="tp")
                for (t, pl, tap, eng) in (
                        (ts, 0, 0, nc.sync),     # Pe0 = Pe[u]
                        (ts, 1, 1, nc.scalar),   # Po1 = Po[u+1]
                        (tp, 0, 2, nc.gpsimd),   # Po0 = Po[u]
                        (tp, 1, 3, nc.sync)):    # Pe1 = Pe[u+1]
                    ap = xin[0:1, 0:1].copy()
                    ap.ap = bass_rust.VecI64Pair([[TAPE, 128], [1, n * W2]])
                    ap.offset = tap * 128 * TAPE + a * W2
                    eng.dma_start(_view(t, 0, n, W2, pl * 7 * W2), ap)
                return ts, tp

            def rep2(t, n):            # one plane broadcast to both parities
                return _viewp(t, 0, 2, 0, n, W2)

            def hsort(tins, a, n):
                ts, tp = tins
                p0 = _view(tp, 0, n, W2)              # Po0
                p1 = _view(tp, 0, n, W2, 7 * W2)      # Pe1
                m = _view(mHe, 0, n, W2)
                Mv = _view(MHe, 0, n, W2)
                nc.vector.tensor_tensor(m, p0, p1, MN)
                nc.vector.tensor_tensor(Mv, p0, p1, MX)
                sg = _viewp(ts, 0, 2, 7 * W2, n, W2)  # {Pe0, Po1}
                nc.vector.tensor_tensor(planes(0, 2, a, n), sg, rep2(mHe, n), MN)
                nc.vector.tensor_tensor(planes(4, 2, a, n), sg, rep2(MHe, n), MX)
                te = planes(2, 2, a, n)
                nc.vector.tensor_tensor(te, sg, rep2(MHe, n), MN)
                nc.vector.tensor_tensor(te, te, rep2(mHe, n), MX)

            def merge(b, C, split_tail=False):
                ne = (C + 1) // 2          # even outputs  r = b, b+2, ..
                no = C // 2                # odd outputs   r = b+1, b+3, ..
                npr = ne                   # pair slots s = b+1, b+3, ..
                # pair regions [PLe,PLo,PXe,PXo,PNe,PNo,PHe,PHo] of 7 rows
                prs = pool.tile([128, 8 * 7 * W2], dt, tag="prs")

                def P(reg, j0, n):         # two-parity pair view at region
                    return _viewp(prs, reg * 7 * W2, 2, 7 * W2, n, W2, W2)

                nc.vector.tensor_tensor(
                    _viewp(prs, 0, 4, 7 * W2, npr, W2),
                    planes(0, 4, b + 1, npr, 2 * W2),
                    planes(0, 4, b + 2, npr, 2 * W2), MX)
                nc.vector.tensor_tensor(
                    _viewp(prs, 4 * 7 * W2, 4, 7 * W2, npr, W2),
                    planes(2, 4, b + 1, npr, 2 * W2),
                    planes(2, 4, b + 2, npr, 2 * W2), MN)

                ml = pool.tile([128, 2 * 13 * W2], dt, tag="ml")
                mh = pool.tile([128, 2 * 13 * W2], dt, tag="mh")
                md = pool.tile([128, 2 * 13 * W2], dt, tag="md")
                tS = pool.tile([128, 2 * 7 * W2], dt, tag="tS")

                def out2(t, base, cnt, nrows=13):   # both parities, rows base, base+2..
                    return _viewp(t, base * W2, 2, nrows * W2, cnt, 2 * W2)

                def tSv(n):                # compact two-parity scratch view
                    return _viewp(tS, 0, 2, 7 * W2, n, W2)

                # even outputs: single = row r, pair slot s = r+1 (index j)
                nc.vector.tensor_tensor(out2(ml, 0, ne), planes(0, 2, b, ne, 2 * W2), P(0, 0, ne), MX)
                nc.vector.tensor_tensor(out2(mh, 0, ne), planes(4, 2, b, ne, 2 * W2), P(6, 0, ne), MN)
                nc.vector.tensor_tensor(tSv(ne), planes(2, 2, b, ne, 2 * W2), P(2, 0, ne), MN)
                nc.vector.tensor_tensor(out2(md, 0, ne), tSv(ne), P(4, 0, ne), MX)
                # odd outputs: single = row r+2, pair slot s = r (index j)
                nc.vector.tensor_tensor(out2(ml, 1, no), P(0, 0, no), planes(0, 2, b + 3, no, 2 * W2), MX)
                nc.vector.tensor_tensor(out2(mh, 1, no), P(6, 0, no), planes(4, 2, b + 3, no, 2 * W2), MN)
                nc.vector.tensor_tensor(tSv(no), planes(2, 2, b + 3, no, 2 * W2), P(2, 0, no), MN)
                nc.vector.tensor_tensor(out2(md, 1, no), tSv(no), P(4, 0, no), MX)

                # final med3(ml, md, mh) -> md; t1 reuses the pairs tile.
                # split_tail: two pieces so the first store overlaps the rest.
                pieces = [(0, C)] if not split_tail else [(0, C // 2), (C // 2, C - C // 2)]
                for (o, n) in pieces:
                    mlv = _viewp(ml, o * W2, 2, 13 * W2, n, W2)
                    mdv = _viewp(md, o * W2, 2, 13 * W2, n, W2)
                    mhv = _viewp(mh, o * W2, 2, 13 * W2, n, W2)
                    t1 = _viewp(prs, o * W2, 2, 13 * W2, n, W2)
                    nc.vector.tensor_tensor(t1, mlv, mdv, MN)
                    nc.vector.tensor_tensor(mlv, mlv, mdv, MX)
                    nc.vector.tensor_tensor(mlv, mlv, mhv, MN)
                    nc.vector.tensor_tensor(mdv, t1, mlv, MX)
                    for par, eng in ((0, nc.sync), (1, nc.scalar)):
                        dst = yout[0:1, 0:1].copy()
                        dst.ap = bass_rust.VecI64Pair(
                            [[R * WO, 128], [WO, n], [1, W2]])
                        dst.offset = (b + o) * WO + par * W2
                        eng.dma_start(dst, _view(md, o, n, W2, par * 13 * W2, W2))

            # software pipeline: loads one chunk ahead of the sorts,
            # merge0 (rows 0..13) after sorts 0-2, merge1 (12..26) last
            tins = {}
            tins[0] = load(*LOAD_CHUNKS[0])
            tins[1] = load(*LOAD_CHUNKS[1])
            hsort(tins.pop(0), *LOAD_CHUNKS[0])
            tins[2] = load(*LOAD_CHUNKS[2])
            hsort(tins.pop(1), *LOAD_CHUNKS[1])
            tins[3] = load(*LOAD_CHUNKS[3])
            hsort(tins.pop(2), *LOAD_CHUNKS[2])
            merge(*MERGE_CHUNKS[0])
            hsort(tins.pop(3), *LOAD_CHUNKS[3])
            merge(*MERGE_CHUNKS[1], split_tail=True)

    nc.compile()
    _CACHE["nc"] = nc
    return nc


def _pack(core_imgs):
    """core_imgs: (IMGS, H, W) -> xin[4*128, 27*256]: per-partition
    contiguous tap planes [Pe0, Po1, Po0, Pe1] of the padded stack."""
    P = np.zeros((IN_ROWS, W + 2), NP_DT)
    for i in range(IMGS):
        r0 = 1 + i * SEP
        P[r0: r0 + H, 1: 1 + W] = core_imgs[i].astype(NP_DT)
    # windows[p, j] = stack row 25p + j  (j = 0..26), overlapping views
    s0, s1 = P.strides
    Wn = np.lib.stride_tricks.as_strided(
        P, shape=(128, INW, W + 2), strides=(R * s0, s0, s1))
    taps = np.empty((4, 128, INW, W2), NP_DT)
    taps[0] = Wn[:, :, 0:2 * W2:2]        # Pe0 = P[2u]
    taps[1] = Wn[:, :, 3:3 + 2 * W2:2]    # Po1 = P[2u+3]
    taps[2] = Wn[:, :, 1:1 + 2 * W2:2]    # Po0 = P[2u+1]
    taps[3] = Wn[:, :, 2:2 + 2 * W2:2]    # Pe1 = P[2u+2]
    return taps.reshape(4 * 128, TAPE)


def _in_maps(noised):
    imgs = np.asarray(noised, dtype=np.float32).reshape(B * CH, H, W)
    return [{"xin": _pack(imgs[c * IMGS:(c + 1) * IMGS])} for c in range(N_CORES)]


def kernel(noised, cover):
    cover = np.asarray(cover)
    nc = _build()
    in_maps = _in_maps(noised)
    res = run_bass_kernel_spmd(nc, in_maps, core_ids=list(range(N_CORES)))
    out = np.empty((B * CH, H, W), np.float32)
    for c in range(N_CORES):
        Y = res.results[c]["yout"]
        for i in range(IMGS):
            Yi = Y[i * SEP: i * SEP + H]
            out[c * IMGS + i, :, 0::2] = Yi[:, :W2].astype(np.float32)
            out[c * IMGS + i, :, 1::2] = Yi[:, W2:].astype(np.float32)
    filtered = out.reshape(B, CH, H, W)
    return filtered, cover


# revision 28
# speedup vs baseline: 1.0562x; 1.0186x over previous
"""3x3 zero-padded median filter (kornia MedianBlur semantics) on 8 trn2 cores.

Input  noised: (16, 3, 512, 512) f32, cover: same shape (pass-through).
Output (filtered, cover) — filtered is float32.

Sharding: pure data parallel over the 48 (B*C) images, 6 images per core.
Host packs each core's 6 images into one zero-separated stack of 3204 rows
(one zero row between/around images gives the vertical zero padding), with
columns DEINTERLEAVED into even/odd planes: row = [Pe (258 cols) | Po
(258 cols)] where Pe[u] = padded-row[2u], Po[u] = padded-row[2u+1].
Partition p owns R=25 consecutive output rows; input window is 27 rows.

All compute is DVE tensor_tensor min/max in fp16 2x mode, every operand
4-byte aligned.  The even/odd split makes the horizontal shared pair
(columns x+1, x+2, shared between triples x and x+1) an aligned operand:

  1. H-sort, 6 instructions per row chunk (5 ops/px):
       mHe=min(Po0,Pe1)  MHe=max(Po0,Pe1)       pair, shared by parities
       {Le,Lo}=min(({Pe0,Po1}), mHe)            fused across parity
       {He,Ho}=max(({Pe0,Po1}), MHe)
       {Me,Mo}=max(min({Pe0,Po1}, MHe), mHe)
  2. V-merge (10 ops/px): median9 = med3(max3(lo), med3(mid), min3(hi))
     over vertical neighbors; vertical pairs at odd row slots are computed
     once and shared by the two adjacent output rows (even output r uses
     pair(r+1), odd r uses pair(r)); pair instructions span 4 planes.

Output is stored column-deinterleaved ([even 256 | odd 256] per row); the
host re-interleaves.  Internal dtype float16 (exact median of fp16-rounded
inputs; output error ~= fp16 rounding, rel err ~2e-4).
"""

import numpy as np

import bass_rust
import concourse.bacc as bacc
import concourse.mybir as mybir
from concourse.tile import TileContext
from concourse.bass_utils import run_bass_kernel_spmd

B, CH, H, W = 16, 3, 512, 512
N_CORES = 8
IMGS = (B * CH) // N_CORES        # 6 images per core
SEP = H + 1                        # 513: image rows + 1 zero separator row
R = 25                             # output rows per partition (128*25 = 3200)
W2 = 256                           # half output width (one parity)
WE = 258                           # stored width of one input parity plane
WP = 2 * WE                        # 516: packed input row width
WO = 512                           # output row width
IN_ROWS = 3204                     # 25*127 + 27 = 3202, zero padded
OUT_ROWS = 128 * R                 # 3200
INW = 27                           # input rows resident per partition

LOAD_CHUNKS = [(0, 3), (3, 6), (9, 6), (15, 6), (21, 6)]   # 27 rows
MERGE_CHUNKS = [(0, 12), (12, 13)]                         # b even, 25 rows
TAPE = INW * W2                                            # one tap-plane block

MN = mybir.AluOpType.min
MX = mybir.AluOpType.max

NP_DT = np.float16

_CACHE = {}

PP = INW * W2                  # plane stride inside the lmh super-tile


def _view(tile, r0, n, width, col0=0, rowstride=W2):
    """AP over `n` rows (stride `rowstride`) of `tile`, cols [col0, col0+width)."""
    ap = tile[:, r0 * rowstride + col0: r0 * rowstride + col0 + width].copy()
    ap.ap = bass_rust.VecI64Pair([list(ap.ap[0]), [rowstride, n], [1, width]])
    return ap


def _viewp(tile, off, nplanes, pstride, n, rowstride, width=W2):
    """4D AP: nplanes planes x n rows x width elems, from elem offset `off`."""
    ap = tile[:, off: off + width].copy()
    ap.ap = bass_rust.VecI64Pair(
        [list(ap.ap[0]), [pstride, nplanes], [rowstride, n], [1, width]])
    return ap


def _build():
    if "nc" in _CACHE:
        return _CACHE["nc"]
    dt = mybir.dt.float16
    nc = bacc.Bacc(enable_partition_id=False)
    # xin row (tap*128 + p) holds tap-plane [27 x 256] of partition p,
    # contiguous, so every load is a single linear burst per partition
    xin = nc.dram_tensor("xin", [4 * 128, TAPE], dt, kind="ExternalInput")
    yout = nc.dram_tensor("yout", [OUT_ROWS, WO], dt, kind="ExternalOutput")

    with TileContext(nc) as tc:
        with tc.tile_pool(name="p", bufs=1) as pool, \
             tc.tile_pool(name="ti", bufs=2) as tip:
            # sorted-column planes [Le, Lo, Me, Mo, He, Ho] in one super-tile
            # so pair/parity ops span planes with one 4D access pattern
            lmh = pool.tile([128, 6 * PP], dt, tag="lmh")
            mHe = pool.tile([128, 7 * W2], dt, tag="mHe")
            MHe = pool.tile([128, 7 * W2], dt, tag="MHe")

            def planes(p0, np_, base, cnt, stride=W2):
                return _viewp(lmh, (p0 * INW + base) * W2, np_, PP, cnt, stride)

            def load(a, n):
                """4 taps of stack rows [a, a+n): Tsingle = [Pe0; Po1],
                Tpair = [Po0; Pe1], each two planes of n x 256.  Issue on
                different queues so descriptor generation overlaps."""
                ts = tip.tile([128, 2 * 7 * W2], dt, tag="ts")
                tp = tip.tile([128, 2 * 7 * W2], dt, tag="tp")
                for (t, pl, tap, eng) in (
                        (tp, 0, 2, nc.sync),     # Po0 = Po[u]    (pair, needed first)
                        (tp, 1, 3, nc.scalar),   # Pe1 = Pe[u+1]
                        (ts, 0, 0, nc.gpsimd),   # Pe0 = Pe[u]
                        (ts, 1, 1, nc.sync)):    # Po1 = Po[u+1]
                    ap = xin[0:1, 0:1].copy()
                    ap.ap = bass_rust.VecI64Pair([[TAPE, 128], [1, n * W2]])
                    ap.offset = tap * 128 * TAPE + a * W2
                    eng.dma_start(_view(t, 0, n, W2, pl * 7 * W2), ap)
                return ts, tp

            def rep2(t, n):            # one plane broadcast to both parities
                return _viewp(t, 0, 2, 0, n, W2)

            def hsort(tins, a, n):
                ts, tp = tins
                p0 = _view(tp, 0, n, W2)              # Po0
                p1 = _view(tp, 0, n, W2, 7 * W2)      # Pe1
                m = _view(mHe, 0, n, W2)
                Mv = _view(MHe, 0, n, W2)
                nc.vector.tensor_tensor(m, p0, p1, MN)
                nc.vector.tensor_tensor(Mv, p0, p1, MX)
                sg = _viewp(ts, 0, 2, 7 * W2, n, W2)  # {Pe0, Po1}
                nc.vector.tensor_tensor(planes(0, 2, a, n), sg, rep2(mHe, n), MN)
                nc.vector.tensor_tensor(planes(4, 2, a, n), sg, rep2(MHe, n), MX)
                te = planes(2, 2, a, n)
                nc.vector.tensor_tensor(te, sg, rep2(MHe, n), MN)
                nc.vector.tensor_tensor(te, te, rep2(mHe, n), MX)

            def merge(b, C, split_tail=False):
                ne = (C + 1) // 2          # even outputs  r = b, b+2, ..
                no = C // 2                # odd outputs   r = b+1, b+3, ..
                npr = ne                   # pair slots s = b+1, b+3, ..
                # pair regions [PLe,PLo,PXe,PXo,PNe,PNo,PHe,PHo] of 7 rows
                prs = pool.tile([128, 8 * 7 * W2], dt, tag="prs")

                def P(reg, j0, n):         # two-parity pair view at region
                    return _viewp(prs, reg * 7 * W2, 2, 7 * W2, n, W2, W2)

                nc.vector.tensor_tensor(
                    _viewp(prs, 0, 4, 7 * W2, npr, W2),
                    planes(0, 4, b + 1, npr, 2 * W2),
                    planes(0, 4, b + 2, npr, 2 * W2), MX)
                nc.vector.tensor_tensor(
                    _viewp(prs, 4 * 7 * W2, 4, 7 * W2, npr, W2),
                    planes(2, 4, b + 1, npr, 2 * W2),
                    planes(2, 4, b + 2, npr, 2 * W2), MN)

                ml = pool.tile([128, 2 * 13 * W2], dt, tag="ml")
                mh = pool.tile([128, 2 * 13 * W2], dt, tag="mh")
                md = pool.tile([128, 2 * 13 * W2], dt, tag="md")
                tS = pool.tile([128, 2 * 7 * W2], dt, tag="tS")

                def out2(t, base, cnt, nrows=13):   # both parities, rows base, base+2..
                    return _viewp(t, base * W2, 2, nrows * W2, cnt, 2 * W2)

                def tSv(n):                # compact two-parity scratch view
                    return _viewp(tS, 0, 2, 7 * W2, n, W2)

                # even outputs: single = row r, pair slot s = r+1 (index j)
                nc.vector.tensor_tensor(out2(ml, 0, ne), planes(0, 2, b, ne, 2 * W2), P(0, 0, ne), MX)
                nc.vector.tensor_tensor(out2(mh, 0, ne), planes(4, 2, b, ne, 2 * W2), P(6, 0, ne), MN)
                nc.vector.tensor_tensor(tSv(ne), planes(2, 2, b, ne, 2 * W2), P(2, 0, ne), MN)
                nc.vector.tensor_tensor(out2(md, 0, ne), tSv(ne), P(4, 0, ne), MX)
                # odd outputs: single = row r+2, pair slot s = r (index j)
                nc.vector.tensor_tensor(out2(ml, 1, no), P(0, 0, no), planes(0, 2, b + 3, no, 2 * W2), MX)
                nc.vector.tensor_tensor(out2(mh, 1, no), P(6, 0, no), planes(4, 2, b + 3, no, 2 * W2), MN)
                nc.vector.tensor_tensor(tSv(no), planes(2, 2, b + 3, no, 2 * W2), P(2, 0, no), MN)
                nc.vector.tensor_tensor(out2(md, 1, no), tSv(no), P(4, 0, no), MX)

                # final med3(ml, md, mh) -> md; t1 reuses the pairs tile.
                # split_tail: two pieces so the first store overlaps the rest.
                pieces = [(0, C)] if not split_tail else [(0, C // 2), (C // 2, C - C // 2)]
                for (o, n) in pieces:
                    mlv = _viewp(ml, o * W2, 2, 13 * W2, n, W2)
                    mdv = _viewp(md, o * W2, 2, 13 * W2, n, W2)
                    mhv = _viewp(mh, o * W2, 2, 13 * W2, n, W2)
                    t1 = _viewp(prs, o * W2, 2, 13 * W2, n, W2)
                    nc.vector.tensor_tensor(t1, mlv, mdv, MN)
                    nc.vector.tensor_tensor(mlv, mlv, mdv, MX)
                    nc.vector.tensor_tensor(mlv, mlv, mhv, MN)
                    nc.vector.tensor_tensor(mdv, t1, mlv, MX)
                    for par, eng in ((0, nc.sync), (1, nc.scalar)):
                        dst = yout[0:1, 0:1].copy()
                        dst.ap = bass_rust.VecI64Pair(
                            [[R * WO, 128], [WO, n], [1, W2]])
                        dst.offset = (b + o) * WO + par * W2
                        eng.dma_start(dst, _view(md, o, n, W2, par * 13 * W2, W2))

            # software pipeline: loads one chunk ahead of the sorts,
            # merge0 (rows 0..13) after sorts 0-2, merge1 (12..26) last
            tins = {}
            tins[0] = load(*LOAD_CHUNKS[0])
            tins[1] = load(*LOAD_CHUNKS[1])
            hsort(tins.pop(0), *LOAD_CHUNKS[0])
            tins[2] = load(*LOAD_CHUNKS[2])
            hsort(tins.pop(1), *LOAD_CHUNKS[1])
            tins[3] = load(*LOAD_CHUNKS[3])
            hsort(tins.pop(2), *LOAD_CHUNKS[2])
            tins[4] = load(*LOAD_CHUNKS[4])
            hsort(tins.pop(3), *LOAD_CHUNKS[3])
            merge(*MERGE_CHUNKS[0])
            hsort(tins.pop(4), *LOAD_CHUNKS[4])
            merge(*MERGE_CHUNKS[1], split_tail=True)

    nc.compile()
    _CACHE["nc"] = nc
    return nc


def _pack(core_imgs):
    """core_imgs: (IMGS, H, W) -> xin[4*128, 27*256]: per-partition
    contiguous tap planes [Pe0, Po1, Po0, Pe1] of the padded stack."""
    P = np.zeros((IN_ROWS, W + 2), NP_DT)
    for i in range(IMGS):
        r0 = 1 + i * SEP
        P[r0: r0 + H, 1: 1 + W] = core_imgs[i].astype(NP_DT)
    # windows[p, j] = stack row 25p + j  (j = 0..26), overlapping views
    s0, s1 = P.strides
    Wn = np.lib.stride_tricks.as_strided(
        P, shape=(128, INW, W + 2), strides=(R * s0, s0, s1))
    taps = np.empty((4, 128, INW, W2), NP_DT)
    taps[0] = Wn[:, :, 0:2 * W2:2]        # Pe0 = P[2u]
    taps[1] = Wn[:, :, 3:3 + 2 * W2:2]    # Po1 = P[2u+3]
    taps[2] = Wn[:, :, 1:1 + 2 * W2:2]    # Po0 = P[2u+1]
    taps[3] = Wn[:, :, 2:2 + 2 * W2:2]    # Pe1 = P[2u+2]
    return taps.reshape(4 * 128, TAPE)


def _in_maps(noised):
    imgs = np.asarray(noised, dtype=np.float32).reshape(B * CH, H, W)
    return [{"xin": _pack(imgs[c * IMGS:(c + 1) * IMGS])} for c in range(N_CORES)]


def kernel(noised, cover):
    cover = np.asarray(cover)
    nc = _build()
    in_maps = _in_maps(noised)
    res = run_bass_kernel_spmd(nc, in_maps, core_ids=list(range(N_CORES)))
    out = np.empty((B * CH, H, W), np.float32)
    for c in range(N_CORES):
        Y = res.results[c]["yout"]
        for i in range(IMGS):
            Yi = Y[i * SEP: i * SEP + H]
            out[c * IMGS + i, :, 0::2] = Yi[:, :W2].astype(np.float32)
            out[c * IMGS + i, :, 1::2] = Yi[:, W2:].astype(np.float32)
    filtered = out.reshape(B, CH, H, W)
    return filtered, cover


# revision 30
# speedup vs baseline: 1.0734x; 1.0162x over previous
"""3x3 zero-padded median filter (kornia MedianBlur semantics) on 8 trn2 cores.

Input  noised: (16, 3, 512, 512) f32, cover: same shape (pass-through).
Output (filtered, cover) — filtered is float32.

Sharding: pure data parallel over the 48 (B*C) images, 6 images per core.
Host packs each core's 6 images into one zero-separated stack of 3204 rows
(one zero row between/around images gives the vertical zero padding), with
columns DEINTERLEAVED even/odd: Pe[u] = padded-row[2u], Po[u] = row[2u+1].
Partition p owns R=25 consecutive output rows; its input window is 27 rows,
shipped as four contiguous per-partition tap planes (Pe0, Po1, Po0, Pe1)
so every DMA load is one linear burst.

All compute is DVE tensor_tensor min/max in fp16 2x mode, every operand
4-byte aligned.  The even/odd split makes the horizontal shared pair
(columns x+1, x+2, shared between triples x and x+1) an aligned operand:

  1. H-sort, 6 instructions per row chunk (5 ops/px):
       mHe=min(Po0,Pe1)  MHe=max(Po0,Pe1)       pair, shared by parities
       {Le,Lo}=min(({Pe0,Po1}), mHe)            fused across parity
       {He,Ho}=max(({Pe0,Po1}), MHe)
       {Me,Mo}=max(min({Pe0,Po1}, MHe), mHe)
  2. V-merge (10 ops/px): median9 = med3(max3(lo), med3(mid), min3(hi))
     over vertical neighbors; vertical pairs at odd row slots are computed
     once and shared by the two adjacent output rows (even output r uses
     pair(r+1), odd r uses pair(r)); pair instructions span 4 planes.

Output is stored column-deinterleaved ([even 256 | odd 256] per row); the
host re-interleaves.  Internal dtype float16 (exact median of fp16-rounded
inputs; output error ~= fp16 rounding, rel err ~2e-4).
"""

import numpy as np

import bass_rust
import concourse.bacc as bacc
import concourse.mybir as mybir
from concourse.tile import TileContext
from concourse.bass_utils import run_bass_kernel_spmd

B, CH, H, W = 16, 3, 512, 512
N_CORES = 8
IMGS = (B * CH) // N_CORES        # 6 images per core
SEP = H + 1                        # 513: image rows + 1 zero separator row
R = 25                             # output rows per partition (128*25 = 3200)
W2 = 256                           # half output width (one parity)
WO = 512                           # output row width
IN_ROWS = 3204                     # 25*127 + 27 = 3202, zero padded
OUT_ROWS = 128 * R                 # 3200
INW = 27                           # input rows resident per partition

LOAD_CHUNKS = [(0, 3), (3, 6), (9, 6), (15, 6), (21, 6)]   # 27 rows
MERGE_CHUNKS = [(0, 12), (12, 13)]                         # b even, 25 rows
TAPE = INW * W2                                            # one tap-plane block

MN = mybir.AluOpType.min
MX = mybir.AluOpType.max

NP_DT = np.float16

_CACHE = {}

PP = INW * W2                  # plane stride inside the lmh super-tile


def _view(tile, r0, n, width, col0=0, rowstride=W2):
    """AP over `n` rows (stride `rowstride`) of `tile`, cols [col0, col0+width)."""
    ap = tile[:, r0 * rowstride + col0: r0 * rowstride + col0 + width].copy()
    ap.ap = bass_rust.VecI64Pair([list(ap.ap[0]), [rowstride, n], [1, width]])
    return ap


def _viewp(tile, off, nplanes, pstride, n, rowstride, width=W2):
    """4D AP: nplanes planes x n rows x width elems, from elem offset `off`."""
    ap = tile[:, off: off + width].copy()
    ap.ap = bass_rust.VecI64Pair(
        [list(ap.ap[0]), [pstride, nplanes], [rowstride, n], [1, width]])
    return ap


def _build():
    if "nc" in _CACHE:
        return _CACHE["nc"]
    dt = mybir.dt.float16
    nc = bacc.Bacc(enable_partition_id=False)
    # xin row (tap*128 + p) holds tap-plane [27 x 256] of partition p,
    # contiguous, so every load is a single linear burst per partition
    xin = nc.dram_tensor("xin", [4 * 128, TAPE], dt, kind="ExternalInput")
    yout = nc.dram_tensor("yout", [OUT_ROWS, WO], dt, kind="ExternalOutput")

    with TileContext(nc) as tc:
        with tc.tile_pool(name="p", bufs=1) as pool, \
             tc.tile_pool(name="ti", bufs=2) as tip:
            # sorted-column planes [Le, Lo, Me, Mo, He, Ho] in one super-tile
            # so pair/parity ops span planes with one 4D access pattern
            lmh = pool.tile([128, 6 * PP], dt, tag="lmh")
            mHe = pool.tile([128, 7 * W2], dt, tag="mHe")
            MHe = pool.tile([128, 7 * W2], dt, tag="MHe")

            def planes(p0, np_, base, cnt, stride=W2):
                return _viewp(lmh, (p0 * INW + base) * W2, np_, PP, cnt, stride)

            def load(a, n):
                """4 taps of stack rows [a, a+n): Tsingle = [Pe0; Po1],
                Tpair = [Po0; Pe1], each two planes of n x 256.  Issue on
                different queues so descriptor generation overlaps."""
                ts = tip.tile([128, 2 * 7 * W2], dt, tag="ts")
                tp = tip.tile([128, 2 * 7 * W2], dt, tag="tp")
                for (t, pl, tap, eng) in (
                        (tp, 0, 2, nc.sync),     # Po0 = Po[u]    (pair, needed first)
                        (tp, 1, 3, nc.scalar),   # Pe1 = Pe[u+1]
                        (ts, 0, 0, nc.gpsimd),   # Pe0 = Pe[u]
                        (ts, 1, 1, nc.sync)):    # Po1 = Po[u+1]
                    ap = xin[0:1, 0:1].copy()
                    ap.ap = bass_rust.VecI64Pair([[TAPE, 128], [1, n * W2]])
                    ap.offset = tap * 128 * TAPE + a * W2
                    eng.dma_start(_view(t, 0, n, W2, pl * 7 * W2), ap)
                return ts, tp

            def rep2(t, n):            # one plane broadcast to both parities
                return _viewp(t, 0, 2, 0, n, W2)

            def hsort(tins, a, n):
                ts, tp = tins
                p0 = _view(tp, 0, n, W2)              # Po0
                p1 = _view(tp, 0, n, W2, 7 * W2)      # Pe1
                m = _view(mHe, 0, n, W2)
                Mv = _view(MHe, 0, n, W2)
                nc.vector.tensor_tensor(m, p0, p1, MN)
                nc.vector.tensor_tensor(Mv, p0, p1, MX)
                sg = _viewp(ts, 0, 2, 7 * W2, n, W2)  # {Pe0, Po1}
                nc.vector.tensor_tensor(planes(0, 2, a, n), sg, rep2(mHe, n), MN)
                nc.vector.tensor_tensor(planes(4, 2, a, n), sg, rep2(MHe, n), MX)
                te = planes(2, 2, a, n)
                nc.vector.tensor_tensor(te, sg, rep2(MHe, n), MN)
                nc.vector.tensor_tensor(te, te, rep2(mHe, n), MX)

            def merge(b, C, split_tail=False):
                ne = (C + 1) // 2          # even outputs  r = b, b+2, ..
                no = C // 2                # odd outputs   r = b+1, b+3, ..
                npr = ne                   # pair slots s = b+1, b+3, ..
                # pair regions [PLe,PLo,PXe,PXo,PNe,PNo,PHe,PHo] of 7 rows
                prs = pool.tile([128, 8 * 7 * W2], dt, tag="prs")

                def P(reg, j0, n):         # two-parity pair view at region
                    return _viewp(prs, reg * 7 * W2, 2, 7 * W2, n, W2, W2)

                nc.vector.tensor_tensor(
                    _viewp(prs, 0, 4, 7 * W2, npr, W2),
                    planes(0, 4, b + 1, npr, 2 * W2),
                    planes(0, 4, b + 2, npr, 2 * W2), MX)
                nc.vector.tensor_tensor(
                    _viewp(prs, 4 * 7 * W2, 4, 7 * W2, npr, W2),
                    planes(2, 4, b + 1, npr, 2 * W2),
                    planes(2, 4, b + 2, npr, 2 * W2), MN)

                ml = pool.tile([128, 2 * 13 * W2], dt, tag="ml")
                mh = pool.tile([128, 2 * 13 * W2], dt, tag="mh")
                md = pool.tile([128, 2 * 13 * W2], dt, tag="md")
                tS = pool.tile([128, 2 * 7 * W2], dt, tag="tS")

                def out2(t, base, cnt, nrows=13):   # both parities, rows base, base+2..
                    return _viewp(t, base * W2, 2, nrows * W2, cnt, 2 * W2)

                def tSv(n):                # compact two-parity scratch view
                    return _viewp(tS, 0, 2, 7 * W2, n, W2)

                # even outputs: single = row r, pair slot s = r+1 (index j)
                nc.vector.tensor_tensor(out2(ml, 0, ne), planes(0, 2, b, ne, 2 * W2), P(0, 0, ne), MX)
                nc.vector.tensor_tensor(out2(mh, 0, ne), planes(4, 2, b, ne, 2 * W2), P(6, 0, ne), MN)
                nc.vector.tensor_tensor(tSv(ne), planes(2, 2, b, ne, 2 * W2), P(2, 0, ne), MN)
                nc.vector.tensor_tensor(out2(md, 0, ne), tSv(ne), P(4, 0, ne), MX)
                # odd outputs: single = row r+2, pair slot s = r (index j)
                nc.vector.tensor_tensor(out2(ml, 1, no), P(0, 0, no), planes(0, 2, b + 3, no, 2 * W2), MX)
                nc.vector.tensor_tensor(out2(mh, 1, no), P(6, 0, no), planes(4, 2, b + 3, no, 2 * W2), MN)
                nc.vector.tensor_tensor(tSv(no), planes(2, 2, b + 3, no, 2 * W2), P(2, 0, no), MN)
                nc.vector.tensor_tensor(out2(md, 1, no), tSv(no), P(4, 0, no), MX)

                # final med3(ml, md, mh) -> md; t1 reuses the pairs tile.
                # split_tail: two pieces so the first store overlaps the rest.
                pieces = [(0, C)] if not split_tail else [(0, C // 2), (C // 2, C - C // 2)]
                for (o, n) in pieces:
                    mlv = _viewp(ml, o * W2, 2, 13 * W2, n, W2)
                    mdv = _viewp(md, o * W2, 2, 13 * W2, n, W2)
                    mhv = _viewp(mh, o * W2, 2, 13 * W2, n, W2)
                    t1 = _viewp(prs, o * W2, 2, 13 * W2, n, W2)
                    nc.vector.tensor_tensor(t1, mlv, mdv, MN)
                    nc.vector.tensor_tensor(mlv, mlv, mdv, MX)
                    nc.vector.tensor_tensor(mlv, mlv, mhv, MN)
                    nc.vector.tensor_tensor(mdv, t1, mlv, MX)
                    for par, eng in ((0, nc.sync), (1, nc.scalar)):
                        dst = yout[0:1, 0:1].copy()
                        dst.ap = bass_rust.VecI64Pair(
                            [[R * WO, 128], [WO, n], [1, W2]])
                        dst.offset = (b + o) * WO + par * W2
                        eng.dma_start(dst, _view(md, o, n, W2, par * 13 * W2, W2))

            # software pipeline: loads one chunk ahead of the sorts,
            # merge0 (rows 0..13) after sorts 0-2, merge1 (12..26) last
            tins = {}
            tins[0] = load(*LOAD_CHUNKS[0])
            tins[1] = load(*LOAD_CHUNKS[1])
            hsort(tins.pop(0), *LOAD_CHUNKS[0])
            tins[2] = load(*LOAD_CHUNKS[2])
            hsort(tins.pop(1), *LOAD_CHUNKS[1])
            tins[3] = load(*LOAD_CHUNKS[3])
            hsort(tins.pop(2), *LOAD_CHUNKS[2])
            tins[4] = load(*LOAD_CHUNKS[4])
            hsort(tins.pop(3), *LOAD_CHUNKS[3])
            merge(*MERGE_CHUNKS[0])
            hsort(tins.pop(4), *LOAD_CHUNKS[4])
            merge(*MERGE_CHUNKS[1], split_tail=True)

    nc.compile()
    _CACHE["nc"] = nc
    return nc


def _pack(core_imgs):
    """core_imgs: (IMGS, H, W) -> xin[4*128, 27*256]: per-partition
    contiguous tap planes [Pe0, Po1, Po0, Pe1] of the padded stack."""
    P = np.zeros((IN_ROWS, W + 2), NP_DT)
    for i in range(IMGS):
        r0 = 1 + i * SEP
        P[r0: r0 + H, 1: 1 + W] = core_imgs[i].astype(NP_DT)
    # windows[p, j] = stack row 25p + j  (j = 0..26), overlapping views
    s0, s1 = P.strides
    Wn = np.lib.stride_tricks.as_strided(
        P, shape=(128, INW, W + 2), strides=(R * s0, s0, s1))
    taps = np.empty((4, 128, INW, W2), NP_DT)
    taps[0] = Wn[:, :, 0:2 * W2:2]        # Pe0 = P[2u]
    taps[1] = Wn[:, :, 3:3 + 2 * W2:2]    # Po1 = P[2u+3]
    taps[2] = Wn[:, :, 1:1 + 2 * W2:2]    # Po0 = P[2u+1]
    taps[3] = Wn[:, :, 2:2 + 2 * W2:2]    # Pe1 = P[2u+2]
    return taps.reshape(4 * 128, TAPE)


def _in_maps(noised):
    imgs = np.asarray(noised, dtype=np.float32).reshape(B * CH, H, W)
    return [{"xin": _pack(imgs[c * IMGS:(c + 1) * IMGS])} for c in range(N_CORES)]


def kernel(noised, cover):
    cover = np.asarray(cover)
    nc = _build()
    in_maps = _in_maps(noised)
    res = run_bass_kernel_spmd(nc, in_maps, core_ids=list(range(N_CORES)))
    out = np.empty((B * CH, H, W), np.float32)
    for c in range(N_CORES):
        Y = res.results[c]["yout"]
        for i in range(IMGS):
            Yi = Y[i * SEP: i * SEP + H]
            out[c * IMGS + i, :, 0::2] = Yi[:, :W2].astype(np.float32)
            out[c * IMGS + i, :, 1::2] = Yi[:, W2:].astype(np.float32)
    filtered = out.reshape(B, CH, H, W)
    return filtered, cover
